# revision 4
# baseline (speedup 1.0000x reference)
"""Trainium2 Bass kernel for nn_Attention_37855841747487 — fp8 DoubleRow version.

Dense transformer attention block: QKV projection, per-head L2-norm with
gamma * sqrt(d), xPos rotary embedding, GQA softmax attention (16 q heads,
4 kv heads), output projection with residual + bias.

Sharding: 8 cores = 2 batches x 4 query-row slices of 512. Each core
computes K/V for its full batch and attention + output projection for its
512 query rows. No collectives.

Key ideas vs the bf16 baseline:
- All five matmul families (KV/Q projections, scores, AV, output
  projection) run in fp8e4m3 with DoubleRow perf mode: pairs of
  128-partition contraction chunks are packed side by side in the free
  dim, giving 0.5 cycles/row = 4x bf16 FLOP throughput. Scores (d=64
  contraction) split d into 2x32 and park each head's k^T/q^T in a
  32-partition band (band = head%4, via matmul tile_position), so even the
  d=64 contraction gets the DoubleRow speedup.
- x ships host-pre-transposed as fp8 (x^T), so there are no on-device
  DMA transposes; the residual ships as fp32 x^T with the output bias
  pre-added; the output is stored transposed and re-transposed on host.
- Softmax: logits are bounded (l2-normalized q/k), so no max pass; the
  denominator comes from a ones-column appended to V, computed from the
  same quantized probabilities as the numerator (errors cancel in the
  ratio). exp is split across ACT (exact exp -> e4m3) and DVE (1-op
  Schraudolph fast-exp: fused multiply-add whose int8-converted result IS
  the e5m2 bit pattern of exp(s/8 - 1)); GPSIMD cannot read PSUM so Pool
  instead carries the SBUF-side norm work (square, most k-rope) and the
  denominator partition-broadcast.
- rope/l2norm run in natural [token, dim] layout; norm commutes with
  rope so the sum-of-squares chain runs parallel to the rotation.
"""

import sys

sys.path.insert(0, "/opt/trn_rl_repo")

import math

import numpy as np

B, N, DIM = 2, 2048, 1024
H, KVH, D = 16, 4, 64
XPOS_SB = 4096
QS = N // 4
NCORES = 8

_CACHE = {}

FE_A = 4.0 * 0.125 / math.log(2.0)          # e5m2 schraudolph slope
FE_B = 60.0 - 4.0 / math.log(2.0) - 0.25    # e5m2 offset, exp(s/8 - 1)
EXP_SHIFT = -1.0                             # p = exp(s/8 - 1)

# per-head engine pattern for the 8 exp chunk-pairs (A=ACT, D=DVE).
# GPSIMD cannot read PSUM on hardware, so Pool cannot run exp.
EXP_PAT = ["AADADADA", "ADAADADA"]


# ---------------------------------------------------------------- host tables
def _make_half_tables(positions, scale_pow, gamma):
    """xPos rotary half-tables with rms folded in. Returns cosT, sinT of
    shape [n, Hg, 32] (bf16-able): full-width tables are the half tables
    duplicated, with the sign of sin handled by sub/add in the kernel."""
    d = D
    half = np.arange(0, d, 2, dtype=np.float64)
    inv_freq = 1.0 / (10000.0 ** (half / d))
    t = positions.astype(np.float64)
    freqs = t[:, None] * inv_freq[None, :]          # [n, 32]
    base_scale = (half + 0.4 * d) / (1.4 * d)
    power = (t - N // 2) / XPOS_SB
    scale = base_scale[None, :] ** power[:, None]   # [n, 32]
    scale = scale**scale_pow
    rms = np.sqrt(np.float64(D))
    g = gamma[:, :32]
    assert np.allclose(gamma[:, :32], gamma[:, 32:]), "gamma halves differ"
    cosT = (np.cos(freqs) * scale)[:, None, :] * (g[None] * rms)
    sinT = (np.sin(freqs) * scale)[:, None, :] * (g[None] * rms)
    return cosT.astype(np.float32), sinT.astype(np.float32)


# ---------------------------------------------------------------- bass kernel
def _build_nc(repeat=1):
    import concourse.bacc as bacc
    import concourse.bass as bass
    import concourse.mybir as mybir
    import concourse.tile as tile
    from concourse.masks import make_identity

    f32 = mybir.dt.float32
    bf16 = mybir.dt.bfloat16
    e4 = mybir.dt.float8e4
    e5 = mybir.dt.float8e5
    i8 = mybir.dt.int8
    AF = mybir.ActivationFunctionType
    AX = mybir.AxisListType
    OP = mybir.AluOpType
    PM = mybir.MatmulPerfMode

    nc = bacc.Bacc("TRN2", target_bir_lowering=False, debug=False,
                   num_devices=NCORES, num_swdge_queues=4)

    xt_d = nc.dram_tensor("xt", [DIM, N], e4, kind="ExternalInput")
    qxt8_d = nc.dram_tensor("qxt8", [DIM, QS], e4, kind="ExternalInput")
    qxt_d = nc.dram_tensor("qxt", [DIM, QS], f32, kind="ExternalInput")
    wq_d = nc.dram_tensor("wq", [DIM, H * D], e4, kind="ExternalInput")
    wkv_d = nc.dram_tensor("wkv", [DIM, 2 * KVH * D], e4, kind="ExternalInput")
    wo_d = nc.dram_tensor("wo", [H * D, DIM], e4, kind="ExternalInput")
    tqc_d = nc.dram_tensor("tqc", [QS, H, 32], bf16, kind="ExternalInput")
    tqs_d = nc.dram_tensor("tqs", [QS, H, 32], bf16, kind="ExternalInput")
    tkc_d = nc.dram_tensor("tkc", [N, KVH, 32], bf16, kind="ExternalInput")
    tks_d = nc.dram_tensor("tks", [N, KVH, 32], bf16, kind="ExternalInput")
    y_d = nc.dram_tensor("y", [DIM, QS], f32, kind="ExternalOutput")

    from contextlib import ExitStack

    with tile.TileContext(nc) as tc, ExitStack() as ctx:
        persist = ctx.enter_context(tc.tile_pool(name="persist", bufs=1))
        stage = ctx.enter_context(tc.tile_pool(name="stage", bufs=4))

        # ---- persistent SBUF tensors
        xt_sb = persist.tile([128, 8, N], e4)            # x^T fp8
        qxt8_sb = persist.tile([128, 8, QS], e4)         # own q rows of x^T
        wq_sb = persist.tile([128, 8, H * D], e4)
        wkv_sb = persist.tile([128, 8, 2 * KVH * D], e4)
        wo_sb = persist.tile([128, 8, DIM], e4)
        qxt_sb = persist.tile([128, 8, QS], f32)         # residual + bias
        kT_sb = persist.tile([128, 2, N], e4)            # band h%4: k^T halves
        qT0_sb = persist.tile([128, 2, 2, QS], e4)
        qT1_sb = persist.tile([128, 2, 2, QS], e4)
        qT_sb = [qT0_sb, qT1_sb]
        v_sb = persist.tile([128, 16, KVH, 2 * D], e4)   # v + ones col + pad
        aoT_sb = persist.tile([128, 8, QS], e4)          # attention out^T
        tqc_sb = persist.tile([128, 4, H, 32], bf16)
        tqs_sb = persist.tile([128, 4, H, 32], bf16)
        tkc_sb = persist.tile([128, 16, KVH, 32], bf16)
        tks_sb = persist.tile([128, 16, KVH, 32], bf16)
        ident_bf = persist.tile([128, 128], bf16)
        expb = persist.tile([128, 1], f32)               # exp bias const
        ones16 = persist.tile([1, D], bf16)              # denominator bcast
        make_identity(nc, ident_bf)
        nc.vector.memset(expb, EXP_SHIFT)
        nc.vector.memset(ones16, 1.0)
        nc.vector.memset(v_sb[:, :, :, D : D + 1], 1.0)
        nc.vector.memset(v_sb[:, :, :, D + 1 : 2 * D], 0.0)
        persist.seal()

        for _rep in range(repeat):
            # ---- A0: stream everything in
            nc.sync.dma_start(out=tkc_sb, in_=tkc_d.rearrange("(a p) h d -> p a h d", p=128))
            nc.sync.dma_start(out=tks_sb, in_=tks_d.rearrange("(a p) h d -> p a h d", p=128))
            for kt in range(8):
                nc.sync.dma_start(out=wkv_sb[:, kt, :],
                                  in_=wkv_d[kt * 128 : (kt + 1) * 128, :])
            xt_r = xt_d.rearrange("(kt p) t -> p kt t", p=128)
            for c in range(4):
                nc.sync.dma_start(out=xt_sb[:, :, c * 512 : (c + 1) * 512],
                                  in_=xt_r[:, :, c * 512 : (c + 1) * 512])
            for kt in range(8):
                nc.sync.dma_start(out=wq_sb[:, kt, :],
                                  in_=wq_d[kt * 128 : (kt + 1) * 128, :])
            nc.sync.dma_start(out=qxt8_sb,
                              in_=qxt8_d.rearrange("(kt p) t -> p kt t", p=128))
            nc.sync.dma_start(out=tqc_sb, in_=tqc_d.rearrange("(a p) h d -> p a h d", p=128))
            nc.sync.dma_start(out=tqs_sb, in_=tqs_d.rearrange("(a p) h d -> p a h d", p=128))

            def norm_rope(pin, cos_t, sin_t, hout2, Hh, ve):
                """pin: PSUM fp32 [128, Hh, 64] projected tile. cos_t/sin_t:
                bf16 [128, Hh, 32] half-tables. hout2: SBUF bf16 HALF-MAJOR
                [128, 2, Hh, 32] = rope(pin)/||pin|| * table-folded gamma*rms,
                with d-half hf of head h at hout2[:, hf, h, :] so that two
                adjacent heads' same-half values are contiguous (lets one PE
                transpose emit a 64-row band pair).
                rope(l2norm(x)) == rope(x)/||x||: the sum-of-squares chain runs
                in parallel with the rotation multiplies and joins at the end.
                ve: engine for the rotation multiplies (nc.vector or nc.gpsimd;
                everything it touches is SBUF so Pool is legal).
                """
                pb = stage.tile([128, Hh, D], bf16, tag="pb")
                nc.scalar.copy(out=pb, in_=pin)
                sq = stage.tile([128, Hh, D], bf16, tag="sq")
                nc.gpsimd.tensor_tensor(out=sq, in0=pb, in1=pb, op=OP.mult)
                ss = stage.tile([128, Hh], f32, tag="ss")
                nc.vector.tensor_reduce(ss, sq, axis=AX.X, op=OP.add)
                nrm = stage.tile([128, Hh], f32, tag="nrm")
                nc.scalar.activation(nrm, ss, AF.Sqrt)
                rs = stage.tile([128, Hh], f32, tag="rs")
                nc.vector.reciprocal(rs, nrm)
                # r1 = pb * cos (half-table broadcast over the two d-halves)
                r1 = stage.tile([128, Hh, D], bf16, tag="t1")
                cb = cos_t.unsqueeze(2).broadcast_to([128, Hh, 2, 32])
                ve.tensor_tensor(
                    out=r1.rearrange("p h (two d) -> p h two d", two=2),
                    in0=pb.rearrange("p h (two d) -> p h two d", two=2),
                    in1=cb, op=OP.mult)
                # t = swapped-half of pb times sin half-table
                t = stage.tile([128, Hh, D], bf16, tag="t2")
                ve.tensor_tensor(out=t[:, :, 0:32], in0=pb[:, :, 32:64],
                                 in1=sin_t, op=OP.mult)
                ve.tensor_tensor(out=t[:, :, 32:64], in0=pb[:, :, 0:32],
                                 in1=sin_t, op=OP.mult)
                # hout2 = (r1 -/+ t) * (1/||pin||), written half-major
                ve.tensor_tensor(out=hout2[:, 0], in0=r1[:, :, 0:32],
                                 in1=t[:, :, 0:32], op=OP.subtract)
                ve.tensor_tensor(out=hout2[:, 1], in0=r1[:, :, 32:64],
                                 in1=t[:, :, 32:64], op=OP.add)
                rsb = rs.unsqueeze(1).unsqueeze(3).broadcast_to([128, 2, Hh, 32])
                ve.tensor_tensor(out=hout2, in0=hout2, in1=rsb, op=OP.mult)

            # ---- A1: kv projection (DoubleRow fp8) + k norm/rope + banded
            # transposes; then q projection likewise.
            with tc.tile_pool(name="kv_ps", bufs=2, space="PSUM") as kv_ps, \
                 tc.tile_pool(name="q_ps", bufs=2, space="PSUM") as q_ps, \
                 tc.tile_pool(name="ktp", bufs=2, space="PSUM") as ktp_ps, \
                 tc.tile_pool(name="qtp", bufs=2, space="PSUM") as qtp_ps:
                for mt in range(16):
                    kvp = kv_ps.tile([128, 2 * KVH * D], f32)
                    for g in range(4):
                        nc.tensor.matmul(
                            kvp,
                            lhsT=xt_sb[:, 2 * g : 2 * g + 2, mt * 128 : (mt + 1) * 128],
                            rhs=wkv_sb[:, 2 * g : 2 * g + 2, :],
                            start=(g == 0), stop=(g == 3), perf_mode=PM.DoubleRow)
                    kv8 = kvp.rearrange("p (g d) -> p g d", d=D)
                    # v evacuation (cast to fp8; ones column pre-set)
                    nc.scalar.copy(out=v_sb[:, mt, :, 0:D],
                                   in_=kv8[:, KVH : 2 * KVH, :])
                    khat = stage.tile([128, 2, KVH, 32], bf16, tag="hat")
                    norm_rope(kv8[:, 0:KVH, :], tkc_sb[:, mt], tks_sb[:, mt],
                              khat, KVH,
                              nc.vector if mt in (2, 5, 8, 11, 14)
                              else nc.gpsimd)
                    # banded split transposes: k head j -> partitions 32j..32j+31,
                    # d-halves side by side in the free dim. Two adjacent heads
                    # share one transpose (half-major khat -> 64-row band pair).
                    ktp = ktp_ps.tile([128, 2, 128], bf16)
                    for jp in (0, 2):
                        for hf in range(2):
                            kf = khat[:, hf].rearrange("p h d -> p (h d)")
                            nc.tensor.transpose(
                                ktp[32 * jp : 32 * jp + 64, hf, :],
                                kf[:, 32 * jp : 32 * jp + 64],
                                ident_bf, tile_position=(0, 32 * jp))
                    nc.scalar.copy(out=kT_sb[:, :, mt * 128 : (mt + 1) * 128],
                                   in_=ktp)

                for nn in range(2):
                    for m in range(4):
                        qp = q_ps.tile([128, 512], f32)
                        for g in range(4):
                            nc.tensor.matmul(
                                qp,
                                lhsT=qxt8_sb[:, 2 * g : 2 * g + 2,
                                             m * 128 : (m + 1) * 128],
                                rhs=wq_sb[:, 2 * g : 2 * g + 2,
                                          nn * 512 : (nn + 1) * 512],
                                start=(g == 0), stop=(g == 3),
                                perf_mode=PM.DoubleRow)
                        qhat = stage.tile([128, 2, 8, 32], bf16, tag="hat")
                        norm_rope(qp.rearrange("p (h d) -> p h d", d=D),
                                  tqc_sb[:, m, nn * 8 : (nn + 1) * 8],
                                  tqs_sb[:, m, nn * 8 : (nn + 1) * 8],
                                  qhat, 8, nc.vector)
                        qtp = qtp_ps.tile([128, 2, 2, 128], bf16)
                        for i in (0, 2, 4, 6):
                            j, sl = i % 4, i // 4
                            for hf in range(2):
                                qf = qhat[:, hf].rearrange("p h d -> p (h d)")
                                nc.tensor.transpose(
                                    qtp[32 * j : 32 * j + 64, sl, hf, :],
                                    qf[:, 32 * i : 32 * i + 64],
                                    ident_bf, tile_position=(0, 32 * j))
                        nc.scalar.copy(
                            out=qT_sb[nn][:, :, :, m * 128 : (m + 1) * 128],
                            in_=qtp)

            # ---- B: attention per head; wo + residual stream in concurrently
            for kt in range(8):
                nc.sync.dma_start(out=wo_sb[:, kt, :],
                                  in_=wo_d[kt * 128 : (kt + 1) * 128, :])
            nc.sync.dma_start(out=qxt_sb,
                              in_=qxt_d.rearrange("(m p) t -> p m t", p=128))

            with tc.tile_pool(name="sT_ps", bufs=3, space="PSUM") as sT_ps, \
                 tc.tile_pool(name="oT_ps", bufs=2, space="PSUM") as oT_ps, \
                 tc.tile_pool(name="pT_pool", bufs=2) as pT_pool, \
                 tc.tile_pool(name="small", bufs=3) as small:
                for h in range(H):
                    j = h % 4          # partition band (== kv head)
                    sl, nn = (h % 8) // 4, h // 8
                    pat = EXP_PAT[h % 2]
                    pT = pT_pool.tile([128, 16, QS], e4)
                    oT = oT_ps.tile([128, QS], f32)
                    pend = None
                    for pr in range(8):
                        eng = pat[pr]
                        sT = sT_ps.tile([128, 2, QS], f32)
                        for i, c in enumerate((2 * pr, 2 * pr + 1)):
                            nc.tensor.matmul(
                                sT[:, i, :],
                                lhsT=kT_sb[32 * j : 32 * j + 32, :,
                                           c * 128 : (c + 1) * 128],
                                rhs=qT_sb[nn][32 * j : 32 * j + 32, sl, :, :],
                                start=True, stop=True, perf_mode=PM.DoubleRow,
                                tile_position=(32 * j, 0))
                        pp = pT[:, 2 * pr : 2 * pr + 2, :]
                        if eng == "A":
                            nc.scalar.activation(pp, sT, AF.Exp, scale=0.125,
                                                 bias=expb[:, 0:1])
                        elif eng == "D":
                            nc.vector.tensor_scalar(
                                out=pp.bitcast(i8), in0=sT,
                                scalar1=FE_A, scalar2=FE_B,
                                op0=OP.mult, op1=OP.add)
                        else:
                            nc.gpsimd.tensor_scalar(
                                out=pp.bitcast(i8), in0=sT,
                                scalar1=FE_A, scalar2=FE_B,
                                op0=OP.mult, op1=OP.add)
                        if pend is not None:
                            pj, peng = pend
                            rhs = pT[:, 2 * pj : 2 * pj + 2, :]
                            nc.tensor.matmul(
                                oT, lhsT=v_sb[:, 2 * pj : 2 * pj + 2, j, :],
                                rhs=rhs if peng == "A" else rhs.bitcast(e5),
                                start=(pj == 0), stop=(pj == 7),
                                perf_mode=PM.DoubleRow)
                        pend = (pr, eng)
                    pj, peng = pend
                    rhs = pT[:, 2 * pj : 2 * pj + 2, :]
                    nc.tensor.matmul(
                        oT, lhsT=v_sb[:, 2 * pj : 2 * pj + 2, j, :],
                        rhs=rhs if peng == "A" else rhs.bitcast(e5),
                        start=(pj == 0), stop=(pj == 7),
                        perf_mode=PM.DoubleRow)
                    recip = small.tile([1, QS], f32, tag="recip")
                    nc.vector.reciprocal(recip, oT[D : D + 1, :])
                    rb = small.tile([D, QS], f32, tag="rb")
                    nc.gpsimd.partition_broadcast(rb, recip)
                    nc.vector.tensor_tensor(
                        out=aoT_sb[64 * (h % 2) : 64 * (h % 2) + 64, h // 2, :],
                        in0=oT[0:D, :], in1=rb, op=OP.mult)

            # ---- C: output projection (DoubleRow fp8) + residual(+bias) in
            # transposed layout; host re-transposes.
            with tc.tile_pool(name="y_ps", bufs=2, space="PSUM") as y_ps, \
                 tc.tile_pool(name="ystage", bufs=2) as ystage:
                for m in range(8):
                    yp = y_ps.tile([128, QS], f32)
                    for g in range(4):
                        nc.tensor.matmul(
                            yp,
                            lhsT=wo_sb[:, 2 * g : 2 * g + 2, m * 128 : (m + 1) * 128],
                            rhs=aoT_sb[:, 2 * g : 2 * g + 2, :],
                            start=(g == 0), stop=(g == 3), perf_mode=PM.DoubleRow)
                    yt = ystage.tile([128, QS], f32)
                    nc.vector.tensor_tensor(out=yt, in0=yp, in1=qxt_sb[:, m, :],
                                            op=OP.add)
                    nc.sync.dma_start(out=y_d[m * 128 : (m + 1) * 128, :], in_=yt)

    nc.compile()
    return nc


def _get_nc(repeat=1):
    if repeat not in _CACHE:
        _CACHE[repeat] = _build_nc(repeat)
    return _CACHE[repeat]


# ---------------------------------------------------------------- entry point
def make_in_maps(x, Wq, Wkv, q_gamma, k_gamma, Wo, bo):
    import ml_dtypes
    bf = ml_dtypes.bfloat16
    E4 = ml_dtypes.float8_e4m3fn
    x = np.ascontiguousarray(np.asarray(x, dtype=np.float32))
    Wq8 = np.ascontiguousarray(np.asarray(Wq, dtype=np.float32).astype(E4))
    Wkv8 = np.ascontiguousarray(np.asarray(Wkv, dtype=np.float32).astype(E4))
    Wo8 = np.ascontiguousarray(np.asarray(Wo, dtype=np.float32).astype(E4))
    bo = np.asarray(bo, dtype=np.float32)
    qg = np.asarray(q_gamma, dtype=np.float64).reshape(H, D)
    kg = np.asarray(k_gamma, dtype=np.float64).reshape(KVH, D)

    pos = np.arange(N)
    tkc, tks = _make_half_tables(pos, -1.0, kg)
    tkc, tks = tkc.astype(bf), tks.astype(bf)

    xt8 = [np.ascontiguousarray(x[bi].T.astype(E4)) for bi in range(B)]
    xtf = [np.ascontiguousarray(x[bi].T) + bo[:, None] for bi in range(B)]

    in_maps = []
    for c in range(NCORES):
        bi, qi = c // 4, c % 4
        qpos = pos[qi * QS : (qi + 1) * QS]
        tqc, tqs = _make_half_tables(qpos, +1.0, qg)
        in_maps.append({
            "xt": xt8[bi],
            "qxt8": np.ascontiguousarray(xt8[bi][:, qi * QS : (qi + 1) * QS]),
            "qxt": np.ascontiguousarray(xtf[bi][:, qi * QS : (qi + 1) * QS]),
            "wq": Wq8, "wkv": Wkv8, "wo": Wo8,
            "tqc": np.ascontiguousarray(tqc.astype(bf)),
            "tqs": np.ascontiguousarray(tqs.astype(bf)),
            "tkc": np.ascontiguousarray(tkc), "tks": np.ascontiguousarray(tks),
        })
    return in_maps


def kernel(x, Wq, Wkv, q_gamma, k_gamma, Wo, bo):
    from concourse import bass_utils

    in_maps = make_in_maps(x, Wq, Wkv, q_gamma, k_gamma, Wo, bo)
    nc = _get_nc()
    res = bass_utils.run_bass_kernel_spmd(nc, in_maps,
                                          core_ids=list(range(NCORES)))
    out = np.zeros((B, N, DIM), np.float32)
    for c in range(NCORES):
        bi, qi = c // 4, c % 4
        out[bi, qi * QS : (qi + 1) * QS] = res.results[c]["y"].T
    return out
